# revision 17
# baseline (speedup 1.0000x reference)
"""GroupedQueryAttention on 8 TRN2 NeuronCores — Bass/Tile kernel.

Sharding: pure data parallel over (batch, query-chunk). Core c handles
batch b = c // 4 and query rows [(c % 4) * 512, (c % 4 + 1) * 512). Each
core computes all 16 heads for its query slice (K/V projections of its
batch are recomputed per core), so outputs are disjoint row slices of
the final [2, 2048, 1024] tensor and no collective is needed.

Device dataflow (everything "transposed": channels on partitions):
  xT   = DMA-transpose(x)                      [D, t]   (bf16, XBAR)
  qpT  = WqT-chunks^T @ xqT  (+bq)             [m, q]   per 128-row m-tile
  kpT  = WkT-chunks^T @ xkT  (+bk)             [m, k]
  vp   = xvT-chunks^T @ WvT  (+bv)             [k, m]   (+ ones column)
  LN over head_dim = 64-partition blocks: stats via selA matmul
  (1/64 block means), invstd = exp(-0.5*ln(var+eps)) on ACT, broadcast
  back via selT matmul, scale/shift via per-partition tensor_scalar.
  S^T  = khat-slice^T @ qhat-slice             [k-chunk, q] per head
  E    = exp(S^T - 20)                         (max-free softmax: LN
         bounds |scores| << 68, so exp never overflows; shift by -20
         gives headroom both ways; the shift cancels in normalization)
  U^T  = vp_aug^T @ E  (accumulated over k)    [65, q]; row 64 = colsum
  out^T= U^T[0:64] * broadcast(1/colsum)       via K=1 matmul broadcast
  z    = concatT-chunks^T @ WoT  (+bo)         [q, m] -> DRAM

Heads are permuted host-side so each head's 64 channels sit at the same
partition base (0 or 64) as its KV group, letting paired [128, ...]
tiles feed base-64 matmuls directly. Wo rows are permuted to match.
"""
import numpy as np
import ml_dtypes

bf16 = ml_dtypes.bfloat16

B = 2
Q = 2048
KV = 2048
DIM = 1024
NH = 16
G = 4
HD = 64
HPG = NH // G
NDEV = 8
QS = (B * Q) // NDEV          # 512 query rows per core
SCALE = 1.0 / np.sqrt(HD)
EPS = 1e-5
SHIFT = 20.0                  # constant exp shift (cancels in softmax)
P = 128

# head placement: pair-tile t, parity par -> head. Parity matches the
# head's KV group parity (g % 2) so khat/qhat partition bases agree.
_A_HEADS = [0, 1, 2, 3, 8, 9, 10, 11]      # groups 0, 2  -> partitions 0-63
_B_HEADS = [4, 5, 6, 7, 12, 13, 14, 15]    # groups 1, 3  -> partitions 64-127


def _head_at(t, par):
    return _A_HEADS[t] if par == 0 else _B_HEADS[t]


def _tile_of(h):
    g, j = h // HPG, h % HPG
    return (g // 2) * HPG + j, g % 2


# channel permutation: new channel c' = t*128 + par*64 + d holds original
# channel head*64 + d
PERM_C = np.empty(DIM, np.int64)
for _t in range(8):
    for _par in range(2):
        _h = _head_at(_t, _par)
        PERM_C[_t * 128 + _par * 64:_t * 128 + _par * 64 + 64] = \
            _h * 64 + np.arange(64)


def _split_waits(nc, mybir, maxw=1):
    """Walrus in this container rejects instructions carrying more than a
    couple of sync-wait commands ("Too many sync wait commands"). Hoist
    excess waits onto same-engine NoOp carriers inserted just before the
    instruction — same-engine program order makes this equivalent."""
    n_nops = 0
    # harvest sem id -> name for the range-clear rewrite below
    sem_names = {}
    for fn in nc.m.functions:
        for bb in fn.blocks:
            for inst in bb.instructions:
                si = inst.sync_info
                if si is None:
                    continue
                for w in (si.on_wait or []):
                    sem_names[w.id] = w.ant_name
                for u in (si.on_update or []):
                    sem_names[u.id] = u.ant_name
    for fn in nc.m.functions:
        for bb in fn.blocks:
            il = bb.instructions
            i = 0
            while i < len(il):
                inst = il[i]
                si = inst.sync_info
                if (inst.__class__.__name__ == "InstISA"
                        and getattr(inst, "op_name", "")
                        == "EVENT_SEMAPHORE_RANGE_CLEAR"):
                    # this walrus build rejects the range-clear ISA payload
                    # ("ISA wrong length") — replace with per-sem wr-imm 0
                    d = inst.ant_dict
                    first, last = d["range_first"], d["range_last"]
                    del il[i]
                    for sem_id in range(first, last + 1):
                        car = mybir.InstEventSemaphore(
                            name=f"semclr-{sem_id}")
                        car.engine = inst.engine
                        upd = mybir.SyncUpdate(
                            sync_type="semaphore", id=sem_id,
                            ant_name=sem_names.get(sem_id, f"sem{sem_id}"),
                            update_mode="sem-wr-imm", update_value=0)
                        car.sync_info = mybir.SyncInfo(on_wait=[],
                                                       on_update=[upd])
                        il.insert(i, car)
                        i += 1
                    continue
                lim = maxw
                if si is not None and si.on_wait and len(si.on_wait) > lim:
                    waits = list(si.on_wait)
                    keep = waits[len(waits) - lim:] if lim else []
                    over = waits[:len(waits) - lim] if lim else waits
                    for w in over:
                        car = mybir.InstEventSemaphore(
                            name=f"{inst.name}-w{n_nops}")
                        n_nops += 1
                        car.engine = inst.engine
                        car.sync_info = mybir.SyncInfo(on_wait=[w],
                                                       on_update=[])
                        il.insert(i, car)
                        i += 1
                    inst.sync_info = mybir.SyncInfo(
                        on_wait=keep, on_update=list(si.on_update))
                i += 1
    return n_nops


def _build_nc(split_waits=True):
    import concourse.bass as bass
    import concourse.mybir as mybir
    import concourse.tile as tile
    from contextlib import ExitStack

    dt = mybir.dt
    alu = mybir.AluOpType
    act = mybir.ActivationFunctionType

    nc = bass.Bass()

    xq_d = nc.dram_tensor("xq", [QS, DIM], dt.bfloat16, kind="ExternalInput")
    xk_d = nc.dram_tensor("xk", [KV, DIM], dt.bfloat16, kind="ExternalInput")
    xv_d = nc.dram_tensor("xv", [KV, DIM], dt.bfloat16, kind="ExternalInput")
    wq_d = nc.dram_tensor("wqT", [DIM, DIM], dt.bfloat16, kind="ExternalInput")
    wk_d = nc.dram_tensor("wkT", [DIM, G * HD], dt.bfloat16, kind="ExternalInput")
    wv_d = nc.dram_tensor("wvT", [DIM, G * HD], dt.bfloat16, kind="ExternalInput")
    wo_d = nc.dram_tensor("woT", [DIM, DIM], dt.bfloat16, kind="ExternalInput")
    bqc_d = nc.dram_tensor("bq_col", [P, 8], dt.float32, kind="ExternalInput")
    bkc_d = nc.dram_tensor("bk_col", [P, 2], dt.float32, kind="ExternalInput")
    lnc_d = nc.dram_tensor("ln_cols", [P, 4], dt.float32, kind="ExternalInput")
    bvt_d = nc.dram_tensor("bv_t", [P, G * HD], dt.float32, kind="ExternalInput")
    bot_d = nc.dram_tensor("bo_t", [P, DIM], dt.float32, kind="ExternalInput")
    out_d = nc.dram_tensor("out", [QS, DIM], dt.float32, kind="ExternalOutput")

    selA_np = np.zeros((P, 2), bf16)
    selA_np[:64, 0] = bf16(1.0 / 64)
    selA_np[64:, 1] = bf16(1.0 / 64)
    selA_d = nc.inline_tensor(selA_np, "selA")
    selT_np = np.zeros((2, P), bf16)
    selT_np[0, :64] = bf16(1.0)
    selT_np[1, 64:] = bf16(1.0)
    selT_d = nc.inline_tensor(selT_np, "selT")
    ident_d = nc.inline_tensor(np.eye(P, dtype=bf16), "ident")

    with tile.TileContext(nc) as tc, ExitStack() as top:
        const = top.enter_context(tc.tile_pool(name="const", bufs=1))

        wq_sb = const.tile([P, 8, DIM], dt.bfloat16)
        nc.sync.dma_start(wq_sb[:], wq_d.rearrange("(o p) m -> p o m", p=P))
        wk_sb = const.tile([P, 8, G * HD], dt.bfloat16)
        nc.sync.dma_start(wk_sb[:], wk_d.rearrange("(o p) m -> p o m", p=P))
        wv_sb = const.tile([P, 8, G * HD], dt.bfloat16)
        nc.sync.dma_start(wv_sb[:], wv_d.rearrange("(o p) m -> p o m", p=P))
        wo_sb = const.tile([P, 8, DIM], dt.bfloat16)
        nc.sync.dma_start(wo_sb[:], wo_d.rearrange("(o p) m -> p o m", p=P))
        bq_col = const.tile([P, 8], dt.float32)
        nc.sync.dma_start(bq_col[:], bqc_d[:])
        bk_col = const.tile([P, 2], dt.float32)
        nc.sync.dma_start(bk_col[:], bkc_d[:])
        ln_cols = const.tile([P, 4], dt.float32)   # qw', qb', kw, kb
        nc.sync.dma_start(ln_cols[:], lnc_d[:])
        bv_t = const.tile([P, G * HD], dt.float32)
        nc.sync.dma_start(bv_t[:], bvt_d[:])
        bo_t = const.tile([P, DIM], dt.float32)
        nc.sync.dma_start(bo_t[:], bot_d[:])
        selA = const.tile([P, 2], dt.bfloat16)
        nc.sync.dma_start(selA[:], selA_d[:])
        selT = const.tile([2, P], dt.bfloat16)
        nc.sync.dma_start(selT[:], selT_d[:])
        ident = const.tile([P, P], dt.bfloat16)
        nc.sync.dma_start(ident[:], ident_d[:])
        ones64 = const.tile([P, 64], dt.bfloat16)
        nc.vector.memset(ones64[:], 1.0)
        epscol = const.tile([P, 1], dt.float32)
        nc.vector.memset(epscol[:], EPS)
        shiftcol = const.tile([P, 1], dt.float32)
        nc.vector.memset(shiftcol[:], -SHIFT)

        # persistent activations
        vp_aug = const.tile([P, 16, G, HD + 1], dt.bfloat16)
        nc.vector.memset(vp_aug[:], 1.0)
        qhat = const.tile([P, 8, QS], dt.bfloat16)     # paired, permuted heads
        khat = const.tile([P, 2, KV], dt.bfloat16)     # paired, natural groups
        concatT = const.tile([P, 8, QS], dt.bfloat16)
        oddtmp = const.tile([64, 8, QS], dt.bfloat16)

        # ---------------- phase 1: transpose + projections + LN ----------
        with tc.tile_pool(name="xpool", bufs=2) as xpool, \
             tc.tile_pool(name="pps", bufs=4, space="PSUM") as pps, \
             tc.tile_pool(name="sps", bufs=4, space="PSUM") as sps, \
             tc.tile_pool(name="stat", bufs=1) as stat, \
             tc.tile_pool(name="work", bufs=3) as work:

            # PE-based transpose of a 512-row slice of x into [d, t]
            # layout: 4 row-strips -> 8 d-chunks x 4 identity matmuls,
            # batched 4-per-psum-bank. (dma_start_transpose is broken in
            # this compiler build: any XPOSE blows the tail-drain's
            # sync-wait limit.)
            def load_transposed(x_dram, t0, tag, idx):
                xT_ch = xpool.tile([P, 8, 512], dt.bfloat16, tag=tag,
                                   bufs=2, name=f"xT_{tag}_{idx}")
                strips = []
                for s in range(4):
                    xs = xpool.tile([P, DIM], dt.bfloat16, tag="xs", bufs=6,
                                    name=f"xs_{tag}_{idx}_{s}")
                    nc.sync.dma_start(xs[:], x_dram[t0 + s * P:
                                                    t0 + (s + 1) * P, :])
                    strips.append(xs)
                for dc in range(8):
                    tp = pps.tile([P, 512], dt.float32, tag="proj",
                                  name=f"tp_{tag}_{idx}_{dc}")
                    for s in range(4):
                        nc.tensor.matmul(tp[:, s * P:(s + 1) * P],
                                         strips[s][:, dc * P:(dc + 1) * P],
                                         ident[:], start=True, stop=True)
                    nc.vector.tensor_copy(xT_ch[:, dc, :], tp[:])
                return xT_ch

            xqT = load_transposed(xq_d, 0, "xq", 0)

            # ---- v projection (k chunks of 512) into vp_aug ----
            for kc in range(KV // 512):
                xvT_ch = load_transposed(xv_d, kc * 512, "xv", kc)
                for ks in range(4):
                    kt = kc * 4 + ks
                    vp_ps = pps.tile([P, 512], dt.float32, tag="proj",
                                     name=f"vp_ps_{kt}")
                    for dc in range(8):
                        nc.tensor.matmul(
                            vp_ps[:, 0:G * HD],
                            xvT_ch[:, dc, ks * P:(ks + 1) * P],
                            wv_sb[:, dc, :], start=(dc == 0), stop=(dc == 7))
                    nc.vector.tensor_tensor(
                        vp_aug[:, kt, :, 0:HD],
                        vp_ps[:, 0:G * HD].rearrange("p (g d) -> p g d", g=G),
                        bv_t[:].rearrange("p (g d) -> p g d", g=G),
                        alu.add)

            # ---- q/k projections + LN, in batches of 4 LN-tiles ----
            # job: (dest [128, 512] slice, psum producer, bias AP, wcol, bcol)
            def q_job(mt):
                def produce(ps_tile):
                    for dc in range(8):
                        nc.tensor.matmul(
                            ps_tile[:], wq_sb[:, dc, mt * P:(mt + 1) * P],
                            xqT[:, dc, :], start=(dc == 0), stop=(dc == 7))
                return (qhat[:, mt, :], produce, bq_col[:, mt:mt + 1],
                        ln_cols[:, 0:1], ln_cols[:, 1:2])

            def k_job(mt2, kc):
                def produce(ps_tile):
                    xkT_ch = get_xk_chunk(kc)
                    for dc in range(8):
                        nc.tensor.matmul(
                            ps_tile[:], wk_sb[:, dc, mt2 * P:(mt2 + 1) * P],
                            xkT_ch[:, dc, :], start=(dc == 0),
                            stop=(dc == 7))
                return (khat[:, mt2, kc * 512:(kc + 1) * 512], produce,
                        bk_col[:, mt2:mt2 + 1],
                        ln_cols[:, 2:3], ln_cols[:, 3:4])

            # k chunks transposed lazily, kc-outer so each chunk dies
            # after both k m-tiles consumed it
            xkT_chs = {}

            def get_xk_chunk(kc):
                if kc not in xkT_chs:
                    xkT_chs[kc] = load_transposed(xk_d, kc * 512, "xk", kc)
                return xkT_chs[kc]

            jobs = [q_job(mt) for mt in range(8)] + \
                   [k_job(mt2, kc) for kc in range(4) for mt2 in range(2)]

            NB = 2                      # LN tiles per stats batch
            for batch in range(len(jobs) // NB):
                bjobs = jobs[batch * NB:(batch + 1) * NB]
                m_b = stat.tile([2, NB, 512], dt.bfloat16, tag="m",
                                name=f"m_b{batch}")
                v_b = stat.tile([2, NB, 512], dt.float32, tag="v",
                                name=f"v_b{batch}")
                for i, (dest, produce, bias, wcol, bcol) in enumerate(bjobs):
                    ps_t = pps.tile([P, 512], dt.float32, tag="proj",
                                    name=f"proj_{batch}_{i}")
                    produce(ps_t)
                    nc.vector.tensor_scalar(dest, ps_t[:], bias, None, alu.add)
                    sq = work.tile([P, 512], dt.bfloat16, tag="sq",
                                   name=f"sq_{batch}_{i}")
                    nc.vector.tensor_tensor(sq[:], dest, dest, alu.mult)
                    st_x = sps.tile([2, 512], dt.float32, tag="st",
                                    name=f"stx_{batch}_{i}")
                    nc.tensor.matmul(st_x[:], selA[:], dest, start=True,
                                     stop=True)
                    st_x2 = sps.tile([2, 512], dt.float32, tag="st",
                                     name=f"stx2_{batch}_{i}")
                    nc.tensor.matmul(st_x2[:], selA[:], sq[:], start=True,
                                     stop=True)
                    nc.vector.tensor_copy(m_b[:, i, :], st_x[:])
                    # m^2 via psum x sbuf (walrus allows only one PSUM input)
                    msq = work.tile([2, 512], dt.float32, tag="msq",
                                    name=f"msq_{batch}_{i}")
                    nc.vector.tensor_tensor(msq[:], st_x[:], m_b[:, i, :],
                                            alu.mult)
                    nc.vector.tensor_tensor(v_b[:, i, :], st_x2[:], msq[:],
                                            alu.subtract)
                lnv = stat.tile([2, NB, 512], dt.float32, tag="lnv",
                                name=f"lnv_b{batch}")
                nc.scalar.activation(lnv[:], v_b[:], act.Ln,
                                     bias=epscol[0:2, :])
                inv_b = stat.tile([2, NB, 512], dt.bfloat16, tag="inv",
                                  name=f"inv_b{batch}")
                nc.scalar.activation(inv_b[:], lnv[:], act.Exp, scale=-0.5)
                for i, (dest, produce, bias, wcol, bcol) in enumerate(bjobs):
                    bc_m = pps.tile([P, 512], dt.float32, tag="proj",
                                    name=f"bcm_{batch}_{i}")
                    nc.tensor.matmul(bc_m[:], selT[:], m_b[:, i, :],
                                     start=True, stop=True)
                    bc_i = pps.tile([P, 512], dt.float32, tag="proj",
                                    name=f"bci_{batch}_{i}")
                    nc.tensor.matmul(bc_i[:], selT[:], inv_b[:, i, :],
                                     start=True, stop=True)
                    u = work.tile([P, 512], dt.bfloat16, tag="u",
                                  name=f"u_{batch}_{i}")
                    nc.vector.tensor_tensor(u[:], dest, bc_m[:], alu.subtract)
                    u2 = work.tile([P, 512], dt.bfloat16, tag="u2",
                                   name=f"u2_{batch}_{i}")
                    nc.vector.tensor_tensor(u2[:], u[:], bc_i[:], alu.mult)
                    nc.vector.tensor_scalar(dest, u2[:], wcol, bcol,
                                            alu.mult, alu.add)

        # ---------------- phase 2: attention --------------------------
        with tc.tile_pool(name="spool", bufs=2, space="PSUM") as spool, \
             tc.tile_pool(name="accp", bufs=4, space="PSUM") as accp, \
             tc.tile_pool(name="ep", bufs=3) as ep, \
             tc.tile_pool(name="nrm", bufs=2) as nrm:
            for g in range(G):
                par = g % 2
                pb = par * 64          # partition base for this group
                gt = g // 2            # khat tile index
                heads = [g * HPG + j for j in range(HPG)]
                tiles = [_tile_of(h)[0] for h in heads]
                accs = [accp.tile([HD + 1, QS], dt.float32, tag="acc",
                                  name=f"acc_{g}_{j}") for j in range(HPG)]
                for kt in range(KV // P):
                    for half in range(2):
                        S = spool.tile([P, 2 * QS], dt.float32, tag="s",
                                       name=f"S_{g}_{kt}_{half}")
                        for jj in range(2):
                            t = tiles[2 * half + jj]
                            nc.tensor.matmul(
                                S[:, jj * QS:(jj + 1) * QS],
                                khat[pb:pb + 64, gt, kt * P:(kt + 1) * P],
                                qhat[pb:pb + 64, t, :],
                                start=True, stop=True)
                        E = ep.tile([P, 2 * QS], dt.bfloat16, tag="e",
                                    name=f"E_{g}_{kt}_{half}")
                        nc.scalar.activation(E[:], S[:], act.Exp,
                                             bias=shiftcol[:])
                        for jj in range(2):
                            j = 2 * half + jj
                            nc.tensor.matmul(
                                accs[j][:], vp_aug[:, kt, g, :],
                                E[:, jj * QS:(jj + 1) * QS],
                                start=(kt == 0), stop=(kt == KV // P - 1))
                for j in range(HPG):
                    t = tiles[j]
                    rec = nrm.tile([65, QS], dt.float32, tag="rec",
                                   name=f"rec_{g}_{j}")
                    nc.vector.reciprocal(rec[64:65, :],
                                         accs[j][64:65, :])
                    recb = nrm.tile([65, QS], dt.bfloat16, tag="recb",
                                    name=f"recb_{g}_{j}")
                    nc.vector.tensor_copy(recb[64:65, :], rec[64:65, :])
                    R = spool.tile([64, QS], dt.float32, tag="s",
                                   name=f"R_{g}_{j}")
                    nc.tensor.matmul(R[:], ones64[64:65, :], recb[64:65, :],
                                     start=True, stop=True)
                    Rs = nrm.tile([64, QS], dt.bfloat16, tag="Rs",
                                  name=f"Rs_{g}_{j}")
                    nc.vector.tensor_copy(Rs[:], R[:])
                    dest = concatT[0:64, t, :] if par == 0 else oddtmp[:, t, :]
                    nc.vector.tensor_tensor(dest, accs[j][0:64, :], Rs[:],
                                            alu.mult)
            nc.sync.dma_start(concatT[64:P, :, :], oddtmp[:])

        # ---------------- phase 3: output projection -------------------
        with tc.tile_pool(name="zps", bufs=3, space="PSUM") as zps, \
             tc.tile_pool(name="zsb", bufs=3) as zsb:
            out_r = out_d.rearrange("(qs p) m -> p qs m", p=P)
            for qs in range(QS // P):
                for mc in range(2):
                    z = zps.tile([P, 512], dt.float32, tag="z",
                                 name=f"z_{qs}_{mc}")
                    for cc in range(8):
                        nc.tensor.matmul(
                            z[:], concatT[:, cc, qs * P:(qs + 1) * P],
                            wo_sb[:, cc, mc * 512:(mc + 1) * 512],
                            start=(cc == 0), stop=(cc == 7))
                    zs = zsb.tile([P, 512], dt.float32, tag="zs",
                                  name=f"zs_{qs}_{mc}")
                    nc.vector.tensor_tensor(zs[:], z[:],
                                            bo_t[:, mc * 512:(mc + 1) * 512],
                                            alu.add)
                    nc.sync.dma_start(out_r[:, qs, mc * 512:(mc + 1) * 512],
                                      zs[:])
    if split_waits:
        _split_waits(nc, mybir, maxw=1)
    return nc


class _Runner:
    def __init__(self):
        import jax
        from jax.experimental.shard_map import shard_map
        from jax.sharding import Mesh, PartitionSpec
        from concourse import bass2jax
        import concourse.mybir as mybir

        bass2jax.install_neuronx_cc_hook()
        nc = self.nc = _build_nc()

        in_names, out_names, out_avals, zero_templates = [], [], [], []
        for alloc in nc.m.functions[0].allocations:
            if not isinstance(alloc, mybir.MemoryLocationSet):
                continue
            name = alloc.memorylocations[0].name
            if alloc.kind == "ExternalInput":
                in_names.append(name)
            elif alloc.kind == "ExternalOutput":
                shape = tuple(alloc.tensor_shape)
                dtype = mybir.dt.np(alloc.dtype)
                out_names.append(name)
                out_avals.append(jax.core.ShapedArray(shape, dtype))
                zero_templates.append((shape, dtype))
        self.in_names = list(in_names)
        self.out_names = out_names
        self.out_avals = out_avals
        self.zero_templates = zero_templates
        n_params = len(in_names)
        n_outs = len(out_names)
        all_in_names = in_names + out_names
        donate = tuple(range(n_params, n_params + n_outs))

        def _body(*args):
            outs = bass2jax._bass_exec_p.bind(
                *args,
                out_avals=tuple(out_avals),
                in_names=tuple(all_in_names),
                out_names=tuple(out_names),
                lowering_input_output_aliases=(),
                sim_require_finite=True,
                sim_require_nnan=True,
                nc=nc,
            )
            return tuple(outs)

        devices = jax.devices()[:NDEV]
        assert len(devices) == NDEV, f"need {NDEV} devices, got {devices}"
        self.mesh = Mesh(np.asarray(devices), ("core",))
        in_specs = (PartitionSpec("core"),) * (n_params + n_outs)
        out_specs = (PartitionSpec("core"),) * n_outs
        self.fn = jax.jit(
            shard_map(_body, mesh=self.mesh, in_specs=in_specs,
                      out_specs=out_specs, check_rep=False),
            donate_argnums=donate, keep_unused=True)

    def make_zeros(self):
        return [np.zeros((NDEV * s[0], *s[1:]), d)
                for s, d in self.zero_templates]

    def concat_inputs(self, in_maps):
        return [np.concatenate([np.asarray(m[n]) for m in in_maps], axis=0)
                for n in self.in_names]

    def run(self, in_maps):
        out = self.fn(*self.concat_inputs(in_maps), *self.make_zeros())
        (shape, dtype) = self.zero_templates[0]
        arr = np.asarray(out[0]).reshape(NDEV, *shape)
        return arr


_RUNNER = None


def _get_runner():
    global _RUNNER
    if _RUNNER is None:
        _RUNNER = _Runner()
    return _RUNNER


def host_prep(query, key, value, Wq, bq, Wk, bk, Wv, bv,
              q_norm_w, q_norm_b, k_norm_w, k_norm_b, Wo, bo):
    """Build the 8 per-core input dicts (host-side slicing/permutation)."""
    f32 = np.float32
    wqT = np.ascontiguousarray(Wq.astype(f32).T[:, PERM_C]).astype(bf16)
    wkT = np.ascontiguousarray(Wk.astype(f32).T).astype(bf16)
    wvT = np.ascontiguousarray(Wv.astype(f32).T).astype(bf16)
    woT = np.ascontiguousarray(Wo.astype(f32).T[PERM_C, :]).astype(bf16)
    bq_col = np.ascontiguousarray(
        bq.astype(f32)[PERM_C].reshape(8, P).T)
    bk_col = np.ascontiguousarray(bk.astype(f32).reshape(2, P).T)
    ln_cols = np.stack([
        np.tile(q_norm_w.astype(f32) * SCALE, 2),
        np.tile(q_norm_b.astype(f32) * SCALE, 2),
        np.tile(k_norm_w.astype(f32), 2),
        np.tile(k_norm_b.astype(f32), 2)], axis=1)
    ln_cols = np.ascontiguousarray(ln_cols)          # [128, 4]
    bv_t = np.ascontiguousarray(
        np.broadcast_to(bv.astype(f32), (P, G * HD)))
    bo_t = np.ascontiguousarray(np.broadcast_to(bo.astype(f32), (P, DIM)))

    qb = query.astype(bf16).reshape(NDEV, QS, DIM)
    kb = key.astype(bf16)
    vb = value.astype(bf16)

    shared = dict(wqT=wqT, wkT=wkT, wvT=wvT, woT=woT, bq_col=bq_col,
                  bk_col=bk_col, ln_cols=ln_cols, bv_t=bv_t, bo_t=bo_t)
    in_maps = []
    for c in range(NDEV):
        b = c // (NDEV // B)
        in_maps.append(dict(xq=np.ascontiguousarray(qb[c]),
                            xk=kb[b], xv=vb[b],
                            partition_id=np.array([[c]], np.uint32),
                            **shared))
    return in_maps


def _reference_fallback(query, key, value, attn_mask, Wq, bq, Wk, bk, Wv, bv,
                        q_norm_w, q_norm_b, k_norm_w, k_norm_b, Wo, bo):
    """Exact float32 numpy reference (only used for non-all-ones masks)."""
    def ln(x, w, b):
        m = x.mean(-1, keepdims=True)
        v = ((x - m) ** 2).mean(-1, keepdims=True)
        return (x - m) / np.sqrt(v + EPS) * w + b

    q = query @ Wq.T + bq
    k = key @ Wk.T + bk
    v = value @ Wv.T + bv
    Bn, Qn = query.shape[0], query.shape[1]
    KVn = key.shape[1]
    q = q.reshape(Bn, Qn, G, HPG, HD).transpose(0, 2, 3, 1, 4)
    k = k.reshape(Bn, KVn, G, HD).transpose(0, 2, 1, 3)
    v = v.reshape(Bn, KVn, G, HD).transpose(0, 2, 1, 3)
    q = ln(q, q_norm_w, q_norm_b)
    k = ln(k, k_norm_w, k_norm_b)
    s = np.einsum('bghqd,bgkd->bghqk', q, k) * SCALE
    s = np.where(attn_mask[:, None, None, :, :], s, np.float32(-3.4e38))
    s = s - s.max(-1, keepdims=True)
    e = np.exp(s)
    a = e / e.sum(-1, keepdims=True)
    o = np.einsum('bghqk,bgkd->bghqd', a, v)
    o = o.transpose(0, 3, 1, 2, 4).reshape(Bn, Qn, DIM)
    return (o @ Wo.T + bo).astype(np.float32)


def kernel(query, key, value, attn_mask, Wq, bq, Wk, bk, Wv, bv,
           q_norm_w, q_norm_b, k_norm_w, k_norm_b, Wo, bo):
    query = np.asarray(query, np.float32)
    key = np.asarray(key, np.float32)
    value = np.asarray(value, np.float32)
    attn_mask = np.asarray(attn_mask, bool)
    args = [np.asarray(a, np.float32) for a in
            (Wq, bq, Wk, bk, Wv, bv, q_norm_w, q_norm_b,
             k_norm_w, k_norm_b, Wo, bo)]

    if (query.shape != (B, Q, DIM) or key.shape != (B, KV, DIM)
            or not attn_mask.all()):
        return _reference_fallback(query, key, value, attn_mask, *args)

    runner = _get_runner()
    in_maps = host_prep(query, key, value, *args)
    parts = runner.run(in_maps)                      # [8, 512, 1024] f32
    return np.ascontiguousarray(
        parts.reshape(B, Q, DIM)).astype(np.float32)
